# revision 22
# baseline (speedup 1.0000x reference)
"""Trainium2 Bass kernel for nn_BatchAllLoss (batch-all margin ranking loss).

Math (reference): for N=2048 anchors with D=128 features, balanced labels
(256 classes x 8 instances, sorted), pairwise euclidean distances
d[i,j] = sqrt(clip(sq_i + sq_j - 2 x_i.x_j, 1e-12)); per anchor the 7
positives (same class, excl. self) and 2040 negatives; outputs:
  loss  = mean relu(margin + pos - neg)    over [N, 7, 2040]
  prec  = mean (neg > pos)                 over [N, 7, 2040]
  pos_mean = mean(pos_dist), neg_mean = mean(neg_dist)

Distribution: anchors sharded over 8 NeuronCores (256 anchors each, as two
128-row chunks).  Each core receives a column-ROTATED copy of X^T
(np.roll by -256*core) so its own anchors sit at columns [0, 256) — this
makes every mask/window offset static and the SPMD program identical on
all cores.  Per-core partial sums [1, 4] are gathered and combined on host
(the all-reduce step), then normalized.

Per-core pipeline (per 128-anchor chunk at rotated column r0):
  PE  : dist^2 into PSUM via two accumulated matmuls per 512-col bank:
        (-2 X_c^T) @ X^T   then   [sq_a; 1]^T @ [1; sq_j]  (K=2 aug trick)
  DVE : clip the 128-col self window at 1e-12 (only place dist^2 can be <=0)
  ACT : dist = Sqrt(psum) with accum_out -> per-row sum of ALL distances
  DVE : extract the 16 8x8 group-diagonal blocks -> PD8[a, m] (pos dists)
        then add +1e30 * blockdiag to the window (masks group cols out)
  per m in 0..7 (the 8 group-relative positive slots):
    hinge: ACT Relu(bias=margin+pd, scale=-1) w/ accum  (or DVE sub+min)
    count: DVE tensor_scalar is_gt w/ accum             (or ACT Sign)
  combine with constant VM weights (self slot m == a%8 excluded), reduce
  across partitions with a ones-matmul -> out[1, 4].
"""

import os
import numpy as np

N, D = 2048, 128
K = 8
NUM_CLASSES = 256
MARGIN = 0.2
BIG = 1e30
NCORES = 8
P = 128
CPC = 2  # chunks (of 128 anchors) per core

# engine split tuning: which m-slots run on which engine
HINGE_DVE_MS = ()       # hinge for these m on DVE (sub+min, negated weights)
COUNT_ACT_MS = ()       # count for these m on ACT (Sign); rest on DVE is_gt

_PROGRAM_CACHE = {}


def _build_masks():
    a = np.arange(P)
    # VM[a, m] = 0 where m == a % 8 (the self slot), else 1
    vm = (np.arange(8)[None, :] != (a % 8)[:, None]).astype(np.float32)
    # blockdiag BD[p, c] = 1 if c // 8 == p // 8
    bd = ((np.arange(P)[None, :] // 8) == (a[:, None] // 8)).astype(np.float32)
    # selector SEL[c, m] = 1 if c % 8 == m  (PD8 = (dist_win*BD)^T-free matmul)
    sel = (np.arange(P)[:, None] % 8 == np.arange(8)[None, :]).astype(np.float32)
    wh = np.zeros((P, 16), np.float32)
    wc = np.zeros((P, 16), np.float32)
    wp = np.zeros((P, 16), np.float32)
    for k in range(CPC):
        for m in range(8):
            col = 8 * k + m
            wh[:, col] = -vm[:, m] if m in HINGE_DVE_MS else vm[:, m]
            wc[:, col] = 0.5 * vm[:, m] if m in COUNT_ACT_MS else vm[:, m]
            wp[:, col] = vm[:, m]
    return bd, sel, wh, wc, wp


def _count_beta_total():
    """Host-side additive constant for the count transform.

    DVE is_gt raw = #gt + 8 (masked cols)        -> beta = -8
    ACT Sign raw  = #gt - #lt + 8; #gt+#lt=2040  -> 0.5*raw + 1016
    Applied per valid (a, m) cell: 112 valid rows per column per core.
    """
    beta = 0.0
    for k in range(CPC):
        for m in range(8):
            b = 1016.0 if m in COUNT_ACT_MS else -8.0
            beta += b * 112.0
    return beta * NCORES


def _build_program(stage=10):
    key = (HINGE_DVE_MS, COUNT_ACT_MS, stage)
    if key in _PROGRAM_CACHE:
        return _PROGRAM_CACHE[key]

    import concourse.bass as bass
    import concourse.bacc as bacc
    import concourse.tile as tile
    import concourse.mybir as mybir

    F32 = mybir.dt.float32
    AF = mybir.ActivationFunctionType
    OP = mybir.AluOpType

    bd, sel, wh, wc, wp = _build_masks()

    nc = bacc.Bacc(
        "TRN2",
        target_bir_lowering=False,
        debug=False,
        enable_asserts=True,
        num_devices=NCORES,
    )
    xt_d = nc.dram_tensor("xt", [P, N], F32, kind="ExternalInput")
    out_d = nc.dram_tensor("out", [1, 4], F32, kind="ExternalOutput")

    cbdb_d = nc.inline_tensor((BIG * bd).astype(np.float32), name="cbdb")
    bd_d = nc.inline_tensor(bd, name="bdm")
    sel_d = nc.inline_tensor(sel, name="sel")
    wh_d = nc.inline_tensor(wh, name="wh")
    wc_d = nc.inline_tensor(wc, name="wc")
    wp_d = nc.inline_tensor(wp, name="wp")
    ones_d = nc.inline_tensor(np.ones((1, N), np.float32), name="onesrow")

    with tile.TileContext(nc) as tc, \
         tc.tile_pool(name="big", bufs=1) as bigp, \
         tc.tile_pool(name="dist", bufs=2) as distp, \
         tc.tile_pool(name="sa", bufs=2) as sap, \
         tc.tile_pool(name="sd", bufs=2) as sdp, \
         tc.tile_pool(name="small", bufs=1) as smallp, \
         tc.tile_pool(name="wm", bufs=2) as wmp, \
         tc.tile_pool(name="pbank", bufs=4, space="PSUM") as pbp, \
         tc.tile_pool(name="psmall", bufs=2, space="PSUM") as psp2:

        # ---- load inputs & constants ----
        xts = bigp.tile([P, N], F32)
        for i in range(16):
            nc.sync.dma_start(out=xts[8 * i:8 * (i + 1), :],
                              in_=xt_d[8 * i:8 * (i + 1), :])
        cbdb = bigp.tile([P, P], F32)
        nc.sync.dma_start(out=cbdb, in_=cbdb_d[:, :])
        bdm = bigp.tile([P, P], F32)
        nc.sync.dma_start(out=bdm, in_=bd_d[:, :])
        sels = bigp.tile([P, 8], F32)
        nc.sync.dma_start(out=sels, in_=sel_d[:, :])
        whs = bigp.tile([P, 16], F32)
        nc.sync.dma_start(out=whs, in_=wh_d[:, :])
        wcs = bigp.tile([P, 16], F32)
        nc.sync.dma_start(out=wcs, in_=wc_d[:, :])
        wps = bigp.tile([P, 16], F32)
        nc.sync.dma_start(out=wps, in_=wp_d[:, :])

        ones2 = smallp.tile([P, 2], F32)
        nc.vector.memset(ones2, 1.0)
        if HINGE_DVE_MS and stage >= 6:
            zeros = bigp.tile([P, N], F32, tag="zeros")
            nc.vector.memset(zeros, 0.0)
        else:
            zeros = None

        # ---- setup: -2*X^T (anchor cols only), X^T**2, sq via ones-matmul ----
        if stage >= 2:
            m2xt = bigp.tile([P, CPC * P], F32)
            for k in range(CPC):
                nc.vector.tensor_scalar(out=m2xt[:, P * k:P * (k + 1)],
                                        in0=xts[:, P * k:P * (k + 1)],
                                        scalar1=-2.0, scalar2=None,
                                        op0=OP.mult)
            xt2 = bigp.tile([P, N], F32)
            nc.vector.tensor_mul(out=xt2, in0=xts, in1=xts)

        # sq row -> augl row0 (per bank), then build aug operands:
        #   augl = [sq; ones], augr = [ones; sq]
        # ones rows + augr row1 go via DMA (engines cannot start at part 1).
        if stage >= 3:
            augl = smallp.tile([2, N], F32)
            augr = smallp.tile([2, N], F32)
            for b in range(4):
                sl = slice(512 * b, 512 * (b + 1))
                sqb = psp2.tile([2, 512], F32, tag="ps")
                nc.tensor.matmul(out=sqb, lhsT=ones2, rhs=xt2[:, sl],
                                 start=True, stop=True)
                nc.vector.tensor_copy(out=augl[0:1, sl], in_=sqb[0:1, :])
            nc.sync.dma_start(out=augl[1:2, :], in_=ones_d[:, :])
            nc.sync.dma_start(out=augr[0:1, :], in_=ones_d[:, :])
            nc.sync.dma_start(out=augr[1:2, :], in_=augl[0:1, :])

        # ---- accumulators over both chunks ----
        if stage >= 5:
            pd8 = smallp.tile([P, 16], F32)
            pdm8 = smallp.tile([P, 16], F32)
        if stage >= 6:
            hs = smallp.tile([P, 16], F32)
        if stage >= 7:
            cs = smallp.tile([P, 16], F32)
        if stage >= 4:
            rs8 = smallp.tile([P, 8], F32)
        if COUNT_ACT_MS and stage >= 5:
            npd8 = smallp.tile([P, 16], F32)
        else:
            npd8 = None

        for k in range(CPC if stage >= 4 else 0):
            r0 = P * k
            dist = distp.tile([P, N], F32, tag="dist")
            for b in range(4):
                sl = slice(512 * b, 512 * (b + 1))
                dq = pbp.tile([P, 512], F32, tag="dq")
                nc.tensor.matmul(out=dq, lhsT=m2xt[:, r0:r0 + P],
                                 rhs=xts[:, sl], start=True, stop=False)
                nc.tensor.matmul(out=dq, lhsT=augl[:, r0:r0 + P],
                                 rhs=augr[:, sl], start=False, stop=True)
                if b == 0:
                    # clip the self window (only place dist^2 can be <= 0)
                    nc.vector.tensor_scalar(out=dq[:, r0:r0 + P],
                                            in0=dq[:, r0:r0 + P],
                                            scalar1=1e-12, scalar2=None,
                                            op0=OP.max)
                nc.scalar.activation(out=dist[:, sl], in_=dq, func=AF.Sqrt,
                                     accum_out=rs8[:, 4 * k + b:4 * k + b + 1])

            if stage < 5:
                continue
            # PD8 via selector matmul on the symmetric masked window:
            # wmask = dist_win * BD;  pd8[a, m] = sum_c wmask[c, a] * sel[c, m]
            # (window block is anchors x anchors -> symmetric up to ~1 ulp)
            wmask = wmp.tile([P, P], F32, tag="wm")
            nc.vector.tensor_mul(out=wmask, in0=dist[:, r0:r0 + P], in1=bdm)
            pd8p = psp2.tile([P, 8], F32, tag="ps")
            nc.tensor.matmul(out=pd8p, lhsT=wmask, rhs=sels,
                             start=True, stop=True)
            nc.vector.tensor_copy(out=pd8[:, 8 * k:8 * k + 8], in_=pd8p)
            # mask group window with +BIG blockdiag
            nc.vector.tensor_tensor(out=dist[:, r0:r0 + P],
                                    in0=dist[:, r0:r0 + P], in1=cbdb,
                                    op=OP.add)
            nc.vector.tensor_scalar(out=pdm8[:, 8 * k:8 * k + 8],
                                    in0=pd8[:, 8 * k:8 * k + 8],
                                    scalar1=MARGIN, scalar2=None, op0=OP.add)
            if npd8 is not None:
                nc.vector.tensor_scalar(out=npd8[:, 8 * k:8 * k + 8],
                                        in0=pd8[:, 8 * k:8 * k + 8],
                                        scalar1=-1.0, scalar2=None,
                                        op0=OP.mult)

            for m in range(8 if stage >= 6 else 0):
                col = 8 * k + m
                if stage < 7 and m in COUNT_ACT_MS:
                    pass
                if m in HINGE_DVE_MS:
                    # accum = sum((dist - c) min 0) = -hinge (WH negates)
                    sd = sdp.tile([P, N], F32, tag="sd")
                    nc.vector.scalar_tensor_tensor(
                        out=sd, in0=dist, scalar=pdm8[:, col:col + 1],
                        in1=zeros, op0=OP.subtract, op1=OP.min,
                        accum_out=hs[:, col:col + 1])
                else:
                    sa = sap.tile([P, N], F32, tag="sa")
                    nc.scalar.activation(out=sa, in_=dist, func=AF.Relu,
                                         bias=pdm8[:, col:col + 1], scale=-1.0,
                                         accum_out=hs[:, col:col + 1])
                if stage < 7:
                    continue
                if m in COUNT_ACT_MS:
                    sa2 = sap.tile([P, N], F32, tag="sa")
                    nc.scalar.activation(out=sa2, in_=dist, func=AF.Sign,
                                         bias=npd8[:, col:col + 1], scale=1.0,
                                         accum_out=cs[:, col:col + 1])
                else:
                    # accum = reduce(out, op1=add, initial=scalar2)
                    sd2 = sdp.tile([P, N], F32, tag="sd")
                    nc.vector.tensor_scalar(out=sd2, in0=dist,
                                            scalar1=pd8[:, col:col + 1],
                                            scalar2=0.0, op0=OP.is_gt,
                                            op1=OP.add,
                                            accum_out=cs[:, col:col + 1])

        # ---- combine ----
        if stage >= 8:
            fin = smallp.tile([P, 4], F32)
            scr_a = smallp.tile([P, 16], F32)
            nc.vector.scalar_tensor_tensor(out=scr_a, in0=hs, scalar=1.0,
                                           in1=whs, op0=OP.mult, op1=OP.mult,
                                           accum_out=fin[:, 0:1])
        if stage >= 9:
            scr_b = smallp.tile([P, 16], F32)
            scr_c = smallp.tile([P, 16], F32)
            scr_d = smallp.tile([P, 16], F32)
            nc.vector.scalar_tensor_tensor(out=scr_b, in0=cs, scalar=1.0,
                                           in1=wcs, op0=OP.mult, op1=OP.mult,
                                           accum_out=fin[:, 1:2])
            nc.vector.scalar_tensor_tensor(out=scr_c, in0=pd8, scalar=1.0,
                                           in1=wps, op0=OP.mult, op1=OP.mult,
                                           accum_out=fin[:, 2:3])
            negpd = smallp.tile([P, 1], F32)
            nc.vector.tensor_scalar(out=scr_d, in0=pd8, scalar1=-1.0,
                                    scalar2=0.0, op0=OP.mult, op1=OP.add,
                                    accum_out=negpd)
            rstot = smallp.tile([P, 1], F32)
            nc.vector.tensor_reduce(out=rstot, in_=rs8,
                                    axis=mybir.AxisListType.X, op=OP.add)
            nc.vector.tensor_add(out=fin[:, 3:4], in0=rstot, in1=negpd)
        if stage >= 10:
            finp = psp2.tile([1, 4], F32, tag="ps")
            nc.tensor.matmul(out=finp, lhsT=ones2[:, 0:1], rhs=fin,
                             start=True, stop=True)
            fout = smallp.tile([1, 4], F32)
            nc.scalar.copy(out=fout, in_=finp)
            nc.sync.dma_start(out=out_d[:, :], in_=fout)
        elif stage >= 8:
            nc.sync.dma_start(out=out_d[:, :], in_=fin[0:1, :])
        else:
            dummy = smallp.tile([1, 4], F32)
            nc.vector.memset(dummy, 0.0)
            nc.sync.dma_start(out=out_d[:, :], in_=dummy)

    nc.compile()
    _PROGRAM_CACHE[key] = nc
    return nc


def _expected_targets():
    return np.repeat(np.arange(NUM_CLASSES, dtype=np.int32), K)


def _numpy_reference(inputs, targets, num_instances):
    """Exact numpy replication of the jax reference (general fallback)."""
    x = np.asarray(inputs, np.float32)
    t = np.asarray(targets)
    n = x.shape[0]
    ni = int(num_instances)
    sq = (x * x).sum(axis=1, dtype=np.float32)
    d2 = sq[:, None] + sq[None, :] - 2.0 * (x @ x.T)
    dist = np.sqrt(np.clip(d2, 1e-12, None)).astype(np.float32)
    same = t[:, None] == t[None, :]
    pos_mask = same & ~np.eye(n, dtype=bool)
    neg_mask = ~same
    pos_idx = np.argsort(~pos_mask, axis=1, kind="stable")[:, : ni - 1]
    neg_idx = np.argsort(~neg_mask, axis=1, kind="stable")[:, : n - ni]
    pos_d = np.take_along_axis(dist, pos_idx, axis=1)
    neg_d = np.take_along_axis(dist, neg_idx, axis=1)
    hinge = np.maximum(MARGIN + pos_d[:, :, None] - neg_d[:, None, :], 0.0)
    loss = np.float32(hinge.mean(dtype=np.float64))
    prec = np.float32(
        (neg_d[:, None, :] > pos_d[:, :, None]).mean(dtype=np.float64))
    return (loss, prec, np.float32(pos_d.mean(dtype=np.float64)),
            np.float32(neg_d.mean(dtype=np.float64)))


def kernel(**inputs):
    x = np.ascontiguousarray(np.asarray(inputs["inputs"], dtype=np.float32))
    targets = np.asarray(inputs["targets"])
    num_instances = int(np.asarray(inputs["num_instances"]))

    if (x.shape != (N, D) or num_instances != K
            or not np.array_equal(targets.astype(np.int64),
                                  _expected_targets().astype(np.int64))):
        return _numpy_reference(x, targets, num_instances)

    from concourse.bass_utils import run_bass_kernel_spmd

    nc = _build_program()
    xt = np.ascontiguousarray(x.T)  # [128, 2048]
    in_maps = []
    for c in range(NCORES):
        s = 256 * c
        rot = np.concatenate([xt[:, s:], xt[:, :s]], axis=1)
        in_maps.append({"xt": np.ascontiguousarray(rot)})

    res = run_bass_kernel_spmd(nc, in_maps, core_ids=list(range(NCORES)))
    fins = np.stack([r["out"].reshape(4) for r in res.results], axis=0)
    tot = fins.sum(axis=0, dtype=np.float64)

    n_pairs = float(N) * (K - 1) * (N - K)
    tot_h, tot_c, tot_p, tot_n = tot
    tot_c = tot_c + _count_beta_total()
    loss = np.float32(tot_h / n_pairs)
    prec = np.float32(tot_c / n_pairs)
    pos_mean = np.float32(tot_p / (float(N) * (K - 1)))
    neg_mean = np.float32(tot_n / (float(N) * (N - K)))
    return loss, prec, pos_mean, neg_mean


if __name__ == "__main__":
    import jax
    import reference as ref
    with jax.default_device(jax.devices("cpu")[0]):
        inp = ref.setup_inputs()
        exp = [float(v) for v in ref.reference(**inp)]
    got = kernel(**{k: np.asarray(v) for k, v in inp.items()})
    for name, e, g in zip(["loss", "prec", "pos_mean", "neg_mean"], exp, got):
        rel = abs(float(g) - e) / max(abs(e), 1e-12)
        print(f"{name}: expected={e:.9g} got={float(g):.9g} rel={rel:.3g}")
